# revision 2
# baseline (speedup 1.0000x reference)
"""TRN2 Bass kernel for NetBackward: X = (I - A_{n-1}/n) @ ... @ (I - A_0/n).

Input  A: [1000, 512, 512] fp32.  Output X: [512, 512] fp32.

Distribution (8 NeuronCores, SPMD), per the contiguous-segment scan strategy:
  - core c gets the contiguous factor segment A[c*125:(c+1)*125]
  - local chain runs in transposed space: Y <- M_i^T @ Y (i descending), so
    the stationary matmul operand is A_i in its natural layout (no
    transposes anywhere).  Matmuls run in fp32r (TF32-like, 11-bit
    mantissa) at full PE rate — ~4x faster than native fp32; A is rounded
    for free by gpsimd cast-DMA during load.
  - accuracy trick: the master state lives as a persistent fp32 PSUM
    accumulator: psum[mb] += A_i^T Y_r_i over all 125 steps, and the
    rounded operand is regenerated each step as
    Y_r = round_fp32r(psum * (-1/n) + I) — one fused DVE op per block.
    Only the A^T(dY) cross term sees rounding, so the partial product is
    fp32-quality (~1e-5) at fp32r speed.
  - the 8 fp32 partials Y_c = P_c^T are AllGathered; every core
    redundantly combines via the same trick on V_{j+1} = V_j + E_j^T V_j
    (E_j = Y_j - I):  X = I + sum_j E_j^T round(V_j) in persistent PSUM.
  - core 0's output is returned.  End-to-end absmax error vs the fp32
    reference: ~5e-5 (the reference's own fp32-vs-fp64 deviation is ~4e-5).
"""

import numpy as np

import concourse.mybir as mybir
from concourse import bacc
from concourse.bass_utils import run_bass_kernel_spmd
from concourse.tile import TileContext

dt = mybir.dt

N = 1000
D = 512
KB = D // 128
NCORES = 8
SEG = N // NCORES
A_BUFS = 6


def _build():
    scale = -1.0 / float(N)
    nc = bacc.Bacc()
    a = nc.dram_tensor("a", [SEG, D, D], dt.float32, kind="ExternalInput")
    out = nc.dram_tensor("out", [D, D], dt.float32, kind="ExternalOutput")

    # blocked identity: eye_blk[p, kb*D + m] = I[kb*128 + p, m]
    eye = np.eye(D, dtype=np.float32)
    eye_blk = eye.reshape(KB, 128, D).transpose(1, 0, 2).reshape(128, KB * D)
    eye_dram = nc.inline_tensor(eye_blk, name="eye_blk")

    y_loc = nc.dram_tensor("y_loc", [D, D], dt.float32)
    y_all = nc.dram_tensor("y_all", [NCORES, D, D], dt.float32, addr_space="Shared")

    a_v = a.rearrange("s (kb p) m -> s p kb m", p=128)

    with TileContext(nc) as tc:
        with (
            tc.tile_pool(name="y", bufs=4) as y_pool,
            tc.tile_pool(name="a", bufs=A_BUFS) as a_pool,
            tc.tile_pool(name="acc", bufs=1, space="PSUM") as acc_pool,
            tc.tile_pool(name="misc", bufs=1) as misc_pool,
        ):
            # constants: identity as fp32r (matmul rhs init) and fp32 (stt in1)
            y_cur_r = misc_pool.tile([128, KB * D], dt.float32r, tag="y0")
            nc.gpsimd.dma_start(out=y_cur_r[:], in_=eye_dram[:])
            eye_f = misc_pool.tile([128, KB * D], dt.float32, tag="eyef")
            nc.sync.dma_start(out=eye_f[:], in_=eye_dram[:])

            # ---- main chain: psum[mb] = sum_i A_i^T Y_r_i  (persistent) ----
            ps_acc = [
                acc_pool.tile([128, D], dt.float32, tag=f"acc{mb}", name=f"acc{mb}")
                for mb in range(KB)
            ]
            for i in range(SEG):
                idx = SEG - 1 - i  # descending factor order
                at = a_pool.tile([128, KB * D], dt.float32r, tag="a", name=f"a{i}")
                at3 = at[:].rearrange("p (kb m) -> p kb m", m=D)
                nc.gpsimd.dma_start(out=at3, in_=a_v[idx])

                y_new_r = y_pool.tile([128, KB * D], dt.float32r, tag="y", name=f"y{i}")
                for mb in range(KB):
                    for kb in range(KB):
                        nc.tensor.matmul(
                            ps_acc[mb][:],
                            at[:, kb * D + 128 * mb : kb * D + 128 * mb + 128],
                            y_cur_r[:, kb * D : (kb + 1) * D],
                            start=(i == 0 and kb == 0),
                            stop=(i == SEG - 1 and kb == KB - 1),
                            skip_group_check=True,
                        )
                    nc.vector.scalar_tensor_tensor(
                        out=y_new_r[:, mb * D : (mb + 1) * D],
                        in0=ps_acc[mb][:],
                        scalar=scale,
                        in1=eye_f[:, mb * D : (mb + 1) * D],
                        op0=mybir.AluOpType.mult,
                        op1=mybir.AluOpType.add,
                    )
                y_cur_r = y_new_r

            # full-precision local partial: Y = psum*scale + I (fp32)
            y_fin = misc_pool.tile([128, KB * D], dt.float32, tag="yfin")
            for mb in range(KB):
                nc.vector.scalar_tensor_tensor(
                    out=y_fin[:, mb * D : (mb + 1) * D],
                    in0=ps_acc[mb][:],
                    scalar=scale,
                    in1=eye_f[:, mb * D : (mb + 1) * D],
                    op0=mybir.AluOpType.mult,
                    op1=mybir.AluOpType.add,
                )

            # ---- AllGather the 8 partials ----
            y_loc_v = y_loc.rearrange("(kb p) m -> p kb m", p=128)
            nc.sync.dma_start(
                out=y_loc_v,
                in_=y_fin[:].rearrange("p (kb m) -> p kb m", m=D),
            )
            nc.gpsimd.collective_compute(
                "AllGather",
                mybir.AluOpType.bypass,
                ins=[y_loc[:]],
                outs=[y_all[:]],
                replica_groups=[list(range(NCORES))],
            )

            # ---- combine: X = I + sum_j E_j^T round(V_j),  E_j = Y_j - I ----
            y_all_v = y_all.rearrange("c (kb p) m -> c p kb m", p=128)
            ps_c = [
                acc_pool.tile([128, D], dt.float32, tag=f"acc{mb}", name=f"cacc{mb}")
                for mb in range(KB)
            ]
            v_r = misc_pool.tile([128, KB * D], dt.float32r, tag="v0")
            nc.gpsimd.dma_start(out=v_r[:], in_=eye_dram[:])
            for j in range(NCORES):
                yjf = a_pool.tile([128, KB * D], dt.float32, tag="a", name=f"yj{j}")
                yj3 = yjf[:].rearrange("p (kb m) -> p kb m", m=D)
                nc.sync.dma_start(out=yj3, in_=y_all_v[j])
                ej = y_pool.tile([128, KB * D], dt.float32r, tag="y", name=f"ej{j}")
                for mb in range(KB):
                    nc.vector.scalar_tensor_tensor(
                        out=ej[:, mb * D : (mb + 1) * D],
                        in0=yjf[:, mb * D : (mb + 1) * D],
                        scalar=1.0,
                        in1=eye_f[:, mb * D : (mb + 1) * D],
                        op0=mybir.AluOpType.mult,
                        op1=mybir.AluOpType.subtract,
                    )
                for mb in range(KB):
                    for kb in range(KB):
                        nc.tensor.matmul(
                            ps_c[mb][:],
                            ej[:, kb * D + 128 * mb : kb * D + 128 * mb + 128],
                            v_r[:, kb * D : (kb + 1) * D],
                            start=(j == 0 and kb == 0),
                            stop=(j == NCORES - 1 and kb == KB - 1),
                            skip_group_check=True,
                        )
                if j < NCORES - 1:
                    v_new = y_pool.tile(
                        [128, KB * D], dt.float32r, tag="y", name=f"vn{j}"
                    )
                    for mb in range(KB):
                        nc.vector.scalar_tensor_tensor(
                            out=v_new[:, mb * D : (mb + 1) * D],
                            in0=ps_c[mb][:],
                            scalar=1.0,
                            in1=eye_f[:, mb * D : (mb + 1) * D],
                            op0=mybir.AluOpType.mult,
                            op1=mybir.AluOpType.add,
                        )
                    v_r = v_new

            x_fin = misc_pool.tile([128, KB * D], dt.float32, tag="xfin")
            for mb in range(KB):
                nc.vector.scalar_tensor_tensor(
                    out=x_fin[:, mb * D : (mb + 1) * D],
                    in0=ps_c[mb][:],
                    scalar=1.0,
                    in1=eye_f[:, mb * D : (mb + 1) * D],
                    op0=mybir.AluOpType.mult,
                    op1=mybir.AluOpType.add,
                )
            out_v = out.rearrange("(kb p) m -> p kb m", p=128)
            nc.sync.dma_start(
                out=out_v,
                in_=x_fin[:].rearrange("p (kb m) -> p kb m", m=D),
            )

    nc.compile()
    return nc


_NC_CACHE = None


def kernel(A: np.ndarray) -> np.ndarray:
    global _NC_CACHE
    A = np.ascontiguousarray(np.asarray(A, dtype=np.float32))
    assert A.shape == (N, D, D), A.shape

    if _NC_CACHE is None:
        _NC_CACHE = _build()
    nc = _NC_CACHE

    in_maps = [{"a": A[c * SEG : (c + 1) * SEG]} for c in range(NCORES)]
    res = run_bass_kernel_spmd(nc, in_maps, list(range(NCORES)))
    return np.asarray(res.results[0]["out"], dtype=np.float32)
